# revision 21
# baseline (speedup 1.0000x reference)
"""ChildSum TreeGRU on 8 Trainium2 NeuronCores — v4.

Data-parallel over trees (16/core). Device math in fp16 (PSUM accumulates
fp32): PE streams 1 col/cycle, DVE tensor_tensor ops hit the packed 2x mode
(tensor_scalar hits 4x), DMA bytes halve. End-to-end fp16 error ~5e-4.

Layout keys:
- Bit-reversed node order within every tree level: children of the parent
  block land as [left | right] contiguous halves, so child-pair ops are
  contiguous tensor_tensor (2x) instead of stride-2 (1x). Bit reversal is
  self-inverse; the host applies it to leaf x and to the output node axis.
- Every on-chip tensor holds BOTH feature halves in one tile, f-major
  ([128p, 2*C]: feature half f at cols [f*C,(f+1)*C)). One DVE instruction
  covers both halves; matmul k/f slices stay contiguous.
- Each level gets its own contiguous tile (tree-major within each f block),
  so children reads are contiguous for PE and DVE at every level.
- PSUM super-batches [f0 nb | f1 nb] (nb<=1024, 4 banks) are filled by both
  output-half matmuls and drained by ONE activation (biases are all zero in
  this problem; a per-half-bias build is kept as fallback), alternating
  between two 4-bank pools.

Structure per core: 4 groups x 4 trees run leaf + lv9 + lv8 + lv7 as a
wavefront (stage buffers double-buffered); levels 6..0 run jointly over all
16 trees. Outputs DMA per level as they complete.
"""
import sys

for p in ("/opt/trn_rl_repo", "/root/.axon_site/_ro/trn_rl_repo"):
    if p not in sys.path:
        sys.path.insert(0, p)

import numpy as np
import concourse.tile as tile
from concourse import bacc, mybir
from concourse.bass_utils import run_bass_kernel_spmd

f32 = mybir.dt.float32
f16 = mybir.dt.float16
AF = mybir.ActivationFunctionType
ALU = mybir.AluOpType

T, DEPTH, NN, H = 128, 11, 2047, 256
NCORES = 8
TPC = T // NCORES          # 16 trees per core
G = 4                      # trees per group
NG = TPC // G              # 4 groups
NLEAF = 1 << (DEPTH - 1)   # 1024
LEAF0 = NLEAF - 1          # 1023
SB = 1024                  # per-half psum super-batch cols (2*SB = 4 banks)
MM = 512                   # one matmul output <= one psum bank


def _emit_level(nc, P, tag, NT, Lpt, hc, out2, Wt, bias, next_ps):
    """One GRU level for NT trees, Lpt parents (Lct=2*Lpt children) per tree.

    hc:   children AP [128, 2*NT*Lct], f-major, tree-major, bit-rev order
    out2: output [128, 2*NT*Lpt] contiguous AP, or list of 2 per-f 3D APs
    """
    Lct = 2 * Lpt
    Lp = NT * Lpt   # parent cols per feature half
    Lc = 2 * Lp     # child cols per feature half

    hc3 = hc.rearrange("p (ft n) -> p ft n", n=Lct)    # [128, 2*NT, Lct]

    # h_sum = h_left + h_right (both feature halves in one op)
    hs = P["hs"].tile([128, 2 * Lp], f16, name=f"hs{tag}", tag="hs")
    nc.vector.tensor_tensor(hs[:], hc3[:, :, 0:Lpt], hc3[:, :, Lpt:Lct], ALU.add)

    r = P["r"].tile([128, 2 * Lp], f16, name=f"r{tag}", tag="r")
    z = P["z"].tile([128, 2 * Lc], f16, name=f"z{tag}", tag="z")
    hcand = P["hc"].tile([128, 2 * Lp], f16, name=f"hcand{tag}", tag="hc")

    def act_batches(kind, wname, src, cols, dst, fn, bias_t):
        # psum super-batches [f0 nb | f1 nb]; one ACT per batch (two with bias)
        d3 = dst[:].rearrange("p (f n) -> p f n", f=2)
        for c0 in range(0, cols, SB):
            nb = min(SB, cols - c0)
            ps = next_ps(2 * nb, f"ps{kind}{tag}_{c0}")
            for f in range(2):
                for s0 in range(0, nb, MM):
                    sn = min(MM, nb - s0)
                    for k in range(2):
                        nc.tensor.matmul(
                            ps[:, f * nb + s0:f * nb + s0 + sn],
                            Wt[wname][k][:, f * 128:(f + 1) * 128],
                            src[:, k * cols + c0 + s0:k * cols + c0 + s0 + sn],
                            start=(k == 0), stop=(k == 1))
            if bias_t is None:
                nc.scalar.activation(d3[:, :, c0:c0 + nb],
                                     ps[:].rearrange("p (f n) -> p f n", f=2), fn)
            else:
                for f in range(2):
                    nc.scalar.activation(d3[:, f:f + 1, c0:c0 + nb],
                                         ps[:, f * nb:(f + 1) * nb], fn,
                                         bias=bias_t[f][:])

    bz, br, bc = (bias["bz"], bias["br"], bias["bc"]) if bias else (None, None, None)
    # z over children and r over h_sum; z first (its DVE chain is longest)
    act_batches("z", "uz", hc, Lc, z, AF.Sigmoid, bz)
    act_batches("r", "ur", hs[:], Lp, r, AF.Sigmoid, br)

    # rh = r * h_sum (in place into hs; hs then feeds the Uc matmul)
    nc.vector.tensor_tensor(hs[:], r[:], hs[:], ALU.mult)
    act_batches("c", "uc", hs[:], Lp, hcand, AF.Tanh, bc)

    z3 = z[:].rearrange("p (ft n) -> p ft n", n=Lct)
    # zs = z_left + z_right - 1  (tensor_scalar runs 4x; stt would be 1x)
    zs = P["zs"].tile([128, 2 * Lp], f16, name=f"zs{tag}", tag="zs")
    nc.vector.tensor_tensor(zs[:], z3[:, :, 0:Lpt], z3[:, :, Lpt:Lct], ALU.add)
    nc.vector.tensor_scalar_sub(zs[:], zs[:], 1.0)
    # zh = z * hc, in place into z
    nc.vector.tensor_tensor(z[:], z[:], hc, ALU.mult)
    # zh_sum = zh_left + zh_right
    zhs = P["zhs"].tile([128, 2 * Lp], f16, name=f"zhs{tag}", tag="zhs")
    nc.vector.tensor_tensor(zhs[:], z3[:, :, 0:Lpt], z3[:, :, Lpt:Lct], ALU.add)
    # t = (zs - 1) * h_cand, in place into hcand
    nc.vector.tensor_tensor(hcand[:], zs[:], hcand[:], ALU.mult)
    # h_new = zh_sum - t
    if isinstance(out2, list):
        for f in range(2):
            nc.vector.tensor_tensor(
                out2[f],
                zhs[:, f * Lp:(f + 1) * Lp].rearrange("p (t n) -> p t n", t=NT),
                hcand[:, f * Lp:(f + 1) * Lp].rearrange("p (t n) -> p t n", t=NT),
                ALU.subtract)
    else:
        nc.vector.tensor_tensor(out2, zhs[:], hcand[:], ALU.subtract)


def _build(use_bias):
    nc = bacc.Bacc("TRN2", debug=False)

    xT_d = nc.dram_tensor("xT", [H, TPC * NLEAF], f16, kind="ExternalInput")
    wT_d = nc.dram_tensor("wT", [H, H], f16, kind="ExternalInput")
    uzT_d = nc.dram_tensor("uzT", [H, H], f16, kind="ExternalInput")
    urT_d = nc.dram_tensor("urT", [H, H], f16, kind="ExternalInput")
    ucT_d = nc.dram_tensor("ucT", [H, H], f16, kind="ExternalInput")
    b_d = {}
    for nm in ("bw", "bz", "br", "bc"):
        b_d[nm] = nc.dram_tensor(nm, [H, 1], f32, kind="ExternalInput")
    hout_d = nc.dram_tensor("h_out", [H, TPC, NN], f16, kind="ExternalOutput")

    with tile.TileContext(nc) as tc:
        from contextlib import ExitStack
        with ExitStack() as ctx:
            P = {}
            P["const"] = ctx.enter_context(tc.tile_pool(name="const", bufs=1))
            P["xg"] = ctx.enter_context(tc.tile_pool(name="xg", bufs=3))
            P["st"] = ctx.enter_context(tc.tile_pool(name="st", bufs=2))
            P["h9"] = ctx.enter_context(tc.tile_pool(name="h9", bufs=2))
            P["h8"] = ctx.enter_context(tc.tile_pool(name="h8", bufs=2))
            P["jl"] = ctx.enter_context(tc.tile_pool(name="jl", bufs=1))
            P["z"] = ctx.enter_context(tc.tile_pool(name="z", bufs=2))
            P["hs"] = ctx.enter_context(tc.tile_pool(name="hs", bufs=2))
            P["r"] = ctx.enter_context(tc.tile_pool(name="r", bufs=2))
            P["zs"] = ctx.enter_context(tc.tile_pool(name="zs", bufs=2))
            P["zhs"] = ctx.enter_context(tc.tile_pool(name="zhs", bufs=2))
            P["hc"] = ctx.enter_context(tc.tile_pool(name="hc", bufs=2))
            P["psA"] = ctx.enter_context(tc.tile_pool(name="psA", bufs=1, space="PSUM"))
            P["psB"] = ctx.enter_context(tc.tile_pool(name="psB", bufs=1, space="PSUM"))

            cp = P["const"]
            Wt = {}
            for nm, d in (("w", wT_d), ("uz", uzT_d), ("ur", urT_d), ("uc", ucT_d)):
                Wt[nm] = [cp.tile([128, H], f16, name=f"{nm}T{k}") for k in range(2)]
                for k in range(2):
                    nc.sync.dma_start(Wt[nm][k][:], d.ap()[k * 128:(k + 1) * 128, :])
            bias = None
            if use_bias:
                bias = {}
                for nm in ("bw", "bz", "br", "bc"):
                    bias[nm] = [cp.tile([128, 1], f32, name=f"{nm}{f}") for f in range(2)]
                    for f in range(2):
                        nc.sync.dma_start(bias[nm][f][:],
                                          b_d[nm].ap()[f * 128:(f + 1) * 128, :])

            psn = [0]

            def next_ps(n, name):
                pl = ("psA", "psB")[psn[0] % 2]
                psn[0] += 1
                return P[pl].tile([128, n], f32, name=name, tag=pl)

            GL = G * NLEAF  # 4096 leaf cols per group per feature half

            def emit_leaf(g):
                gt = f"g{g}"
                st = P["st"].tile([128, 2 * GL], f16, name=f"st{gt}", tag="st")
                st3 = st[:].rearrange("p (f n) -> p f n", f=2)
                for c0 in range(0, GL, SB):
                    xt = P["xg"].tile([128, 2 * SB], f16, name=f"x{gt}_{c0}", tag="x")
                    for k in range(2):
                        nc.sync.dma_start(
                            xt[:, k * SB:(k + 1) * SB],
                            xT_d.ap()[k * 128:(k + 1) * 128,
                                      g * GL + c0:g * GL + c0 + SB])
                    ps = next_ps(2 * SB, f"psx{gt}_{c0}")
                    for f in range(2):
                        for s0 in range(0, SB, MM):
                            for k in range(2):
                                nc.tensor.matmul(
                                    ps[:, f * SB + s0:f * SB + s0 + MM],
                                    Wt["w"][k][:, f * 128:(f + 1) * 128],
                                    xt[:, k * SB + s0:k * SB + s0 + MM],
                                    start=(k == 0), stop=(k == 1))
                    if bias is None:
                        nc.scalar.activation(st3[:, :, c0:c0 + SB],
                                             ps[:].rearrange("p (f n) -> p f n", f=2),
                                             AF.Tanh)
                    else:
                        for f in range(2):
                            nc.scalar.activation(st3[:, f:f + 1, c0:c0 + SB],
                                                 ps[:, f * SB:(f + 1) * SB], AF.Tanh,
                                                 bias=bias["bw"][f][:])
                for f in range(2):
                    nc.sync.dma_start(
                        hout_d.ap()[f * 128:(f + 1) * 128, g * G:(g + 1) * G,
                                    LEAF0:LEAF0 + NLEAF],
                        st[:, f * GL:(f + 1) * GL].rearrange("p (t n) -> p t n", t=G))
                return st

            # joint per-level tiles for levels 7..0 (all 16 trees)
            jl = {}
            for lv in range(7, -1, -1):
                jl[lv] = P["jl"].tile([128, 2 * TPC * (1 << lv)], f16,
                                      name=f"jl{lv}", tag=f"jl{lv}")

            def emit_lvl(g, lv, hchild):
                gt = f"g{g}"
                Lpt = 2 ** lv
                if lv == 7:
                    jw = TPC * Lpt
                    out2 = [jl[7][:, f * jw:(f + 1) * jw]
                            .rearrange("p (t n) -> p t n", t=TPC)
                            [:, g * G:(g + 1) * G, :] for f in range(2)]
                    hnew = None
                else:
                    pool = {9: "h9", 8: "h8"}[lv]
                    hnew = P[pool].tile([128, 2 * G * Lpt], f16,
                                        name=f"h{lv}{gt}", tag=pool)
                    out2 = hnew[:]
                _emit_level(nc, P, f"{gt}l{lv}", G, Lpt, hchild, out2, Wt, bias,
                            next_ps)
                if lv > 7:
                    for f in range(2):
                        nc.sync.dma_start(
                            hout_d.ap()[f * 128:(f + 1) * 128, g * G:(g + 1) * G,
                                        Lpt - 1:2 * Lpt - 1],
                            hnew[:, f * G * Lpt:(f + 1) * G * Lpt]
                            .rearrange("p (t n) -> p t n", t=G))
                return hnew

            # wavefront: stage s (0=leaf, 1=lv9, 2=lv8, 3=lv7) of group g at
            # tick g+s, deepest stage emitted first within a tick
            gstate = {}
            for t in range(NG + 3):
                for s in (3, 2, 1, 0):
                    g = t - s
                    if not (0 <= g < NG):
                        continue
                    if s == 0:
                        gstate[g] = emit_leaf(g)
                    else:
                        gstate[g] = emit_lvl(g, 10 - s, gstate[g][:])

            for f in range(2):
                jw = TPC * 128
                nc.sync.dma_start(
                    hout_d.ap()[f * 128:(f + 1) * 128, :, 127:255],
                    jl[7][:, f * jw:(f + 1) * jw].rearrange("p (t n) -> p t n", t=TPC))

            # joint levels 6..0 over all 16 trees
            for lv in range(6, -1, -1):
                Lpt = 2 ** lv
                _emit_level(nc, P, f"j{lv}", TPC, Lpt, jl[lv + 1][:], jl[lv][:],
                            Wt, bias, next_ps)
                jw = TPC * Lpt
                for f in range(2):
                    nc.sync.dma_start(
                        hout_d.ap()[f * 128:(f + 1) * 128, :, Lpt - 1:2 * Lpt - 1],
                        jl[lv][:, f * jw:(f + 1) * jw]
                        .rearrange("p (t n) -> p t n", t=TPC))

    nc.compile()
    return nc


_NC = None
_NC_BIAS = None


def _get_nc(use_bias):
    global _NC, _NC_BIAS
    if use_bias:
        if _NC_BIAS is None:
            _NC_BIAS = _build(True)
        return _NC_BIAS
    if _NC is None:
        _NC = _build(False)
    return _NC


def _bitrev(n_bits):
    idx = np.arange(1 << n_bits)
    rev = np.zeros(1 << n_bits, dtype=np.int64)
    for b in range(n_bits):
        rev |= ((idx >> b) & 1) << (n_bits - 1 - b)
    return rev


def _node_perm():
    """perm[stored_node] = natural_node; self-inverse (bit reversal)."""
    perm = np.empty(NN, dtype=np.int64)
    for l in range(DEPTH):
        base = (1 << l) - 1
        perm[base:base + (1 << l)] = base + _bitrev(l)
    return perm


_PERM = _node_perm()
_LEAF_REV = _bitrev(DEPTH - 1)


def make_in_maps(inputs):
    x = np.asarray(inputs["x"], np.float32)
    W = np.asarray(inputs["W"], np.float32)
    bW = np.asarray(inputs["bW"], np.float32).reshape(H, 1)
    Ur = np.asarray(inputs["Ur"], np.float32)
    br = np.asarray(inputs["br"], np.float32).reshape(H, 1)
    Uc = np.asarray(inputs["Uc"], np.float32)
    bc = np.asarray(inputs["bc"], np.float32).reshape(H, 1)
    Uz = np.asarray(inputs["Uz"], np.float32)
    bz = np.asarray(inputs["bz"], np.float32).reshape(H, 1)
    shared = {
        "wT": np.ascontiguousarray(W.T).astype(np.float16),
        "uzT": np.ascontiguousarray(Uz.T).astype(np.float16),
        "urT": np.ascontiguousarray(Ur.T).astype(np.float16),
        "ucT": np.ascontiguousarray(Uc.T).astype(np.float16),
        "bw": bW, "bz": bz, "br": br, "bc": bc,
    }
    in_maps = []
    for c in range(NCORES):
        # leaves in bit-reversed storage order: stored col p = leaf rev(p)
        xs = x[c * TPC:(c + 1) * TPC, LEAF0 + _LEAF_REV, :]   # [16, 1024, 256]
        xTc = np.ascontiguousarray(xs.transpose(2, 0, 1)).reshape(H, TPC * NLEAF)
        in_maps.append({"xT": xTc.astype(np.float16), **shared})
    return in_maps


def assemble_out(core_outs):
    out = np.empty((T, NN, H), np.float32)
    for c in range(NCORES):
        # [256, 16, 2047] (stored order) -> [16, 2047, 256] natural order
        out[c * TPC:(c + 1) * TPC] = np.asarray(
            core_outs[c], np.float32).transpose(1, 2, 0)[:, _PERM, :]
    return out


def kernel(**inputs):
    use_bias = any(
        np.any(np.asarray(inputs[k], np.float32) != 0.0)
        for k in ("bW", "br", "bc", "bz"))
    nc = _get_nc(use_bias)
    in_maps = make_in_maps(inputs)
    res = run_bass_kernel_spmd(nc, in_maps, list(range(NCORES)))
    return assemble_out([r["h_out"] for r in res.results])


# revision 22
# speedup vs baseline: 1.1687x; 1.1687x over previous
"""ChildSum TreeGRU on 8 Trainium2 NeuronCores — v5.

Data-parallel over trees (16/core). Device math in fp16 (PSUM accumulates
fp32): PE streams 1 col/cycle, DVE tensor_tensor ops hit the packed 2x mode
(tensor_scalar hits 4x), DMA bytes halve. End-to-end fp16 error ~5e-4.

Layout keys:
- Bit-reversed node order within every tree level: children of the parent
  block land as [left | right] contiguous halves, so child-pair ops are
  contiguous tensor_tensor (2x) instead of stride-2 (1x). Bit reversal is
  self-inverse; the host applies it to leaf x and to the output node axis.
- Every level lives in its own contiguous per-feature-half tile (tree-major),
  so matmul moving operands and DVE reads are contiguous at every level.
- Ops stay per-feature-half (two parallel chains per level) — finer
  dependency granularity schedules better than merged-half ops.

Structure per core: 4 groups x 4 trees run leaf + lv9 + lv8 + lv7 as a
wavefront (stage tiles double-buffered, deepest stage emitted first);
levels 6..0 run jointly over all 16 trees. PSUM: two 4-bank pools of
2048-col batches, z/r batches interleaved across pools.
"""
import sys

for p in ("/opt/trn_rl_repo", "/root/.axon_site/_ro/trn_rl_repo"):
    if p not in sys.path:
        sys.path.insert(0, p)

import numpy as np
import concourse.tile as tile
from concourse import bacc, mybir
from concourse.bass_utils import run_bass_kernel_spmd

f32 = mybir.dt.float32
f16 = mybir.dt.float16
AF = mybir.ActivationFunctionType
ALU = mybir.AluOpType

T, DEPTH, NN, H = 128, 11, 2047, 256
NCORES = 8
TPC = T // NCORES          # 16 trees per core
G = 4                      # trees per group
NG = TPC // G              # 4 groups
NLEAF = 1 << (DEPTH - 1)   # 1024
LEAF0 = NLEAF - 1          # 1023
PSB = 2048                 # psum batch (4 banks) consumed by one ACT
MM = 512                   # one matmul output <= one psum bank


def _emit_level(nc, P, tag, NT, Lpt, hc, out2, Wt, bias, next_ps):
    """One GRU level for NT trees, Lpt parents (Lct=2*Lpt children) per tree.

    hc:   per-half children APs [128, NT*Lct], tree-major, bit-rev order
    out2: per-half output APs [128, NT*Lpt] (2D contiguous or 3D view)
    """
    Lct = 2 * Lpt
    Lp = NT * Lpt
    Lc = 2 * Lp
    uzT, urT, ucT = Wt["uz"], Wt["ur"], Wt["uc"]
    bz, br, bc = (bias["bz"], bias["br"], bias["bc"]) if bias else (None, None, None)

    hc3 = [hc[m].rearrange("p (t n) -> p t n", t=NT) for m in range(2)]

    # h_sum = h_left + h_right, contiguous blocks per tree
    hs = [P["hs"].tile([128, Lp], f16, name=f"hs{tag}_{m}", tag=f"hs{m}") for m in range(2)]
    for m in range(2):
        nc.vector.tensor_tensor(hs[m][:], hc3[m][:, :, 0:Lpt], hc3[m][:, :, Lpt:Lct],
                                ALU.add)

    r = [P["r"].tile([128, Lp], f16, name=f"r{tag}_{m}", tag=f"r{m}") for m in range(2)]
    z = [P["z"].tile([128, Lc], f16, name=f"z{tag}_{m}", tag=f"z{m}") for m in range(2)]
    hcand = [P["hc"].tile([128, Lp], f16, name=f"hcand{tag}_{m}", tag=f"hcand{m}")
             for m in range(2)]

    def mm_batch(ps, lhs_pair, rhs, c0, n):
        for s0 in range(0, n, MM):
            sn = min(MM, n - s0)
            nc.tensor.matmul(ps[:, s0:s0 + sn], lhs_pair[0],
                             rhs[0][:, c0 + s0:c0 + s0 + sn], start=True, stop=False)
            nc.tensor.matmul(ps[:, s0:s0 + sn], lhs_pair[1],
                             rhs[1][:, c0 + s0:c0 + s0 + sn], start=False, stop=True)

    # z (over children) and r (over h_sum) psum batches interleaved across the
    # two psum pools: ACT drains one pool while PE fills the other
    batches = []
    zoff = [0, 0]
    roff = [0, 0]
    while min(zoff) < Lc or min(roff) < Lp:
        for m in range(2):
            if zoff[m] < Lc:
                n = min(PSB, Lc - zoff[m])
                batches.append(("z", m, zoff[m], n))
                zoff[m] += n
            if roff[m] < Lp:
                n = min(PSB, Lp - roff[m])
                batches.append(("r", m, roff[m], n))
                roff[m] += n
    for kind, m, c0, n in batches:
        ps = next_ps(n, f"ps{kind}{tag}_{m}_{c0}")
        if kind == "z":
            lhs = [uzT[k][:, m * 128:(m + 1) * 128] for k in range(2)]
            mm_batch(ps, lhs, hc, c0, n)
            nc.scalar.activation(z[m][:, c0:c0 + n], ps[:], AF.Sigmoid,
                                 **({"bias": bz[m][:]} if bias else {}))
        else:
            lhs = [urT[k][:, m * 128:(m + 1) * 128] for k in range(2)]
            mm_batch(ps, lhs, hs, c0, n)
            nc.scalar.activation(r[m][:, c0:c0 + n], ps[:], AF.Sigmoid,
                                 **({"bias": br[m][:]} if bias else {}))

    # rh = r * h_sum (in place into hs; hs then feeds the Uc matmul)
    for m in range(2):
        nc.vector.tensor_tensor(hs[m][:], r[m][:], hs[m][:], ALU.mult)

    # h_cand = tanh(Uc @ rh + bc)
    for c0 in range(0, Lp, PSB):
        n = min(PSB, Lp - c0)
        for m in range(2):
            ps = next_ps(n, f"psc{tag}_{m}_{c0}")
            lhs = [ucT[k][:, m * 128:(m + 1) * 128] for k in range(2)]
            mm_batch(ps, lhs, hs, c0, n)
            nc.scalar.activation(hcand[m][:, c0:c0 + n], ps[:], AF.Tanh,
                                 **({"bias": bc[m][:]} if bias else {}))

    for m in range(2):
        z3 = z[m][:].rearrange("p (t n) -> p t n", t=NT)
        # zs = z_left + z_right - 1 (tensor_scalar runs 4x; stt would be 1x)
        zs = P["zs"].tile([128, Lp], f16, name=f"zs{tag}_{m}", tag=f"zs{m}")
        nc.vector.tensor_tensor(zs[:], z3[:, :, 0:Lpt], z3[:, :, Lpt:Lct], ALU.add)
        nc.vector.tensor_scalar_sub(zs[:], zs[:], 1.0)
        # zh = z * hc, in place into z
        nc.vector.tensor_tensor(z[m][:], z[m][:], hc[m], ALU.mult)
        # zh_sum = zh_left + zh_right
        zhs = P["zhs"].tile([128, Lp], f16, name=f"zhs{tag}_{m}", tag=f"zhs{m}")
        nc.vector.tensor_tensor(zhs[:], z3[:, :, 0:Lpt], z3[:, :, Lpt:Lct], ALU.add)
        # t = (zs - 1) * h_cand, in place into hcand
        nc.vector.tensor_tensor(hcand[m][:], zs[:], hcand[m][:], ALU.mult)
        # h_new = zh_sum - t
        nc.vector.tensor_tensor(out2[m], zhs[:], hcand[m][:], ALU.subtract)


def _build(use_bias):
    nc = bacc.Bacc("TRN2", debug=False)

    xT_d = nc.dram_tensor("xT", [H, TPC * NLEAF], f16, kind="ExternalInput")
    wT_d = nc.dram_tensor("wT", [H, H], f16, kind="ExternalInput")
    uzT_d = nc.dram_tensor("uzT", [H, H], f16, kind="ExternalInput")
    urT_d = nc.dram_tensor("urT", [H, H], f16, kind="ExternalInput")
    ucT_d = nc.dram_tensor("ucT", [H, H], f16, kind="ExternalInput")
    b_d = {}
    for nm in ("bw", "bz", "br", "bc"):
        b_d[nm] = nc.dram_tensor(nm, [H, 1], f32, kind="ExternalInput")
    hout_d = nc.dram_tensor("h_out", [H, TPC, NN], f16, kind="ExternalOutput")

    with tile.TileContext(nc) as tc:
        from contextlib import ExitStack
        with ExitStack() as ctx:
            P = {}
            P["const"] = ctx.enter_context(tc.tile_pool(name="const", bufs=1))
            P["xg"] = ctx.enter_context(tc.tile_pool(name="xg", bufs=2))
            P["st"] = ctx.enter_context(tc.tile_pool(name="st", bufs=2))
            P["h9"] = ctx.enter_context(tc.tile_pool(name="h9", bufs=2))
            P["h8"] = ctx.enter_context(tc.tile_pool(name="h8", bufs=2))
            P["jl"] = ctx.enter_context(tc.tile_pool(name="jl", bufs=1))
            P["z"] = ctx.enter_context(tc.tile_pool(name="z", bufs=2))
            P["hs"] = ctx.enter_context(tc.tile_pool(name="hs", bufs=2))
            P["r"] = ctx.enter_context(tc.tile_pool(name="r", bufs=2))
            P["zs"] = ctx.enter_context(tc.tile_pool(name="zs", bufs=2))
            P["zhs"] = ctx.enter_context(tc.tile_pool(name="zhs", bufs=2))
            P["hc"] = ctx.enter_context(tc.tile_pool(name="hc", bufs=2))
            P["psA"] = ctx.enter_context(tc.tile_pool(name="psA", bufs=1, space="PSUM"))
            P["psB"] = ctx.enter_context(tc.tile_pool(name="psB", bufs=1, space="PSUM"))

            cp = P["const"]
            Wt = {}
            for nm, d in (("w", wT_d), ("uz", uzT_d), ("ur", urT_d), ("uc", ucT_d)):
                Wt[nm] = [cp.tile([128, H], f16, name=f"{nm}T{k}") for k in range(2)]
                for k in range(2):
                    nc.sync.dma_start(Wt[nm][k][:], d.ap()[k * 128:(k + 1) * 128, :])
            bias = None
            if use_bias:
                bias = {}
                for nm in ("bw", "bz", "br", "bc"):
                    bias[nm] = [cp.tile([128, 1], f32, name=f"{nm}{m}") for m in range(2)]
                    for m in range(2):
                        nc.sync.dma_start(bias[nm][m][:],
                                          b_d[nm].ap()[m * 128:(m + 1) * 128, :])

            psn = [0]

            def next_ps(n, name):
                pl = ("psA", "psB")[psn[0] % 2]
                psn[0] += 1
                return P[pl].tile([128, n], f32, name=name, tag=pl)

            GL = G * NLEAF  # 4096 leaf cols per group

            def emit_leaf(g):
                gt = f"g{g}"
                st = [P["st"].tile([128, GL], f16, name=f"st{gt}_{m}", tag=f"st{m}")
                      for m in range(2)]
                for c0 in range(0, GL, PSB):
                    xt = [P["xg"].tile([128, PSB], f16, name=f"x{gt}_{c0}_{k}", tag=f"x{k}")
                          for k in range(2)]
                    for k in range(2):
                        nc.sync.dma_start(
                            xt[k][:], xT_d.ap()[k * 128:(k + 1) * 128,
                                                g * GL + c0:g * GL + c0 + PSB])
                    for m in range(2):
                        ps = next_ps(PSB, f"psx{gt}_{m}_{c0}")
                        lhs = [Wt["w"][k][:, m * 128:(m + 1) * 128] for k in range(2)]
                        for s0 in range(0, PSB, MM):
                            nc.tensor.matmul(ps[:, s0:s0 + MM], lhs[0],
                                             xt[0][:, s0:s0 + MM], start=True, stop=False)
                            nc.tensor.matmul(ps[:, s0:s0 + MM], lhs[1],
                                             xt[1][:, s0:s0 + MM], start=False, stop=True)
                        nc.scalar.activation(st[m][:, c0:c0 + PSB], ps[:], AF.Tanh,
                                             **({"bias": bias["bw"][m][:]} if bias else {}))
                for m in range(2):
                    nc.sync.dma_start(
                        hout_d.ap()[m * 128:(m + 1) * 128, g * G:(g + 1) * G,
                                    LEAF0:LEAF0 + NLEAF],
                        st[m][:].rearrange("p (t n) -> p t n", t=G))
                return st

            # joint per-level tiles for levels 7..0 (all 16 trees, per half)
            jl = {}
            for lv in range(7, -1, -1):
                jl[lv] = [P["jl"].tile([128, TPC * (1 << lv)], f16,
                                       name=f"jl{lv}_{m}", tag=f"jl{lv}_{m}")
                          for m in range(2)]

            def emit_lvl(g, lv, hchild):
                gt = f"g{g}"
                Lpt = 2 ** lv
                if lv == 7:
                    out2 = [jl[7][m][:].rearrange("p (t n) -> p t n", t=TPC)
                            [:, g * G:(g + 1) * G, :] for m in range(2)]
                    hnew = None
                else:
                    pool = {9: "h9", 8: "h8"}[lv]
                    hnew = [P[pool].tile([128, G * Lpt], f16,
                                         name=f"h{lv}{gt}_{m}", tag=f"{pool}{m}")
                            for m in range(2)]
                    out2 = [hnew[m][:] for m in range(2)]
                _emit_level(nc, P, f"{gt}l{lv}", G, Lpt,
                            [hchild[m][:] for m in range(2)], out2, Wt, bias, next_ps)
                if lv > 7:
                    for m in range(2):
                        nc.sync.dma_start(
                            hout_d.ap()[m * 128:(m + 1) * 128, g * G:(g + 1) * G,
                                        Lpt - 1:2 * Lpt - 1],
                            hnew[m][:].rearrange("p (t n) -> p t n", t=G))
                return hnew

            # wavefront: stage s (0=leaf, 1=lv9, 2=lv8, 3=lv7) of group g at
            # tick g+s, deepest stage emitted first within a tick
            gstate = {}
            for t in range(NG + 3):
                for s in (3, 2, 1, 0):
                    g = t - s
                    if not (0 <= g < NG):
                        continue
                    if s == 0:
                        gstate[g] = emit_leaf(g)
                    else:
                        gstate[g] = emit_lvl(g, 10 - s, gstate[g])

            for m in range(2):
                nc.sync.dma_start(
                    hout_d.ap()[m * 128:(m + 1) * 128, :, 127:255],
                    jl[7][m][:].rearrange("p (t n) -> p t n", t=TPC))

            # joint levels 6..0 over all 16 trees
            for lv in range(6, -1, -1):
                Lpt = 2 ** lv
                _emit_level(nc, P, f"j{lv}", TPC, Lpt,
                            [jl[lv + 1][m][:] for m in range(2)],
                            [jl[lv][m][:] for m in range(2)], Wt, bias, next_ps)
                for m in range(2):
                    nc.sync.dma_start(
                        hout_d.ap()[m * 128:(m + 1) * 128, :, Lpt - 1:2 * Lpt - 1],
                        jl[lv][m][:].rearrange("p (t n) -> p t n", t=TPC))

    nc.compile()
    return nc


_NC = None
_NC_BIAS = None


def _get_nc(use_bias):
    global _NC, _NC_BIAS
    if use_bias:
        if _NC_BIAS is None:
            _NC_BIAS = _build(True)
        return _NC_BIAS
    if _NC is None:
        _NC = _build(False)
    return _NC


def _bitrev(n_bits):
    idx = np.arange(1 << n_bits)
    rev = np.zeros(1 << n_bits, dtype=np.int64)
    for b in range(n_bits):
        rev |= ((idx >> b) & 1) << (n_bits - 1 - b)
    return rev


def _node_perm():
    """perm[stored_node] = natural_node; self-inverse (bit reversal)."""
    perm = np.empty(NN, dtype=np.int64)
    for l in range(DEPTH):
        base = (1 << l) - 1
        perm[base:base + (1 << l)] = base + _bitrev(l)
    return perm


_PERM = _node_perm()
_LEAF_REV = _bitrev(DEPTH - 1)


def make_in_maps(inputs):
    x = np.asarray(inputs["x"], np.float32)
    W = np.asarray(inputs["W"], np.float32)
    bW = np.asarray(inputs["bW"], np.float32).reshape(H, 1)
    Ur = np.asarray(inputs["Ur"], np.float32)
    br = np.asarray(inputs["br"], np.float32).reshape(H, 1)
    Uc = np.asarray(inputs["Uc"], np.float32)
    bc = np.asarray(inputs["bc"], np.float32).reshape(H, 1)
    Uz = np.asarray(inputs["Uz"], np.float32)
    bz = np.asarray(inputs["bz"], np.float32).reshape(H, 1)
    shared = {
        "wT": np.ascontiguousarray(W.T).astype(np.float16),
        "uzT": np.ascontiguousarray(Uz.T).astype(np.float16),
        "urT": np.ascontiguousarray(Ur.T).astype(np.float16),
        "ucT": np.ascontiguousarray(Uc.T).astype(np.float16),
        "bw": bW, "bz": bz, "br": br, "bc": bc,
    }
    in_maps = []
    for c in range(NCORES):
        # leaves in bit-reversed storage order: stored col p = leaf rev(p)
        xs = x[c * TPC:(c + 1) * TPC, LEAF0 + _LEAF_REV, :]   # [16, 1024, 256]
        xTc = np.ascontiguousarray(xs.transpose(2, 0, 1)).reshape(H, TPC * NLEAF)
        in_maps.append({"xT": xTc.astype(np.float16), **shared})
    return in_maps


def assemble_out(core_outs):
    out = np.empty((T, NN, H), np.float32)
    for c in range(NCORES):
        # [256, 16, 2047] (stored order) -> [16, 2047, 256] natural order
        out[c * TPC:(c + 1) * TPC] = np.asarray(
            core_outs[c], np.float32).transpose(1, 2, 0)[:, _PERM, :]
    return out


def kernel(**inputs):
    use_bias = any(
        np.any(np.asarray(inputs[k], np.float32) != 0.0)
        for k in ("bW", "br", "bc", "bz"))
    nc = _get_nc(use_bias)
    in_maps = make_in_maps(inputs)
    res = run_bass_kernel_spmd(nc, in_maps, list(range(NCORES)))
    return assemble_out([r["h_out"] for r in res.results])
